# revision 9
# baseline (speedup 1.0000x reference)
"""CopyNet extended-vocab projection kernel for Trainium2 (8 NeuronCores).

out[b, t, v] = p_gen[b,t] * pad(dist_t)[b,t,v] + (1 - p_gen[b,t]) * copyp[b,t,v]
copyp[b, t, v] = sum_{s: pointer[b,s]==v} alph_t[b, s, t]

Strategy: pure data-parallel over batch (B=8 -> 8 cores, one batch element per
core). The kernel is HBM-bound, so all bulk streams are bf16 (the rel-err
budget is 2e-2; the full bf16 chain is ~6e-3 worst-case): the host ships dist
and alpha as bf16, the device writes a bf16 output, and the host upcasts to
f32. Per-core HBM traffic is 16.4 MB (dist in) + 16.5 MB (out) + 0.3 MB
(alpha) ~= 33 MB vs 66 MB for the all-f32 variant.

Per core the output streams through SBUF in 4096-wide vocab macro-tiles; the
copy term is a one-hot matmul on the tensor engine: onehot[s, v] =
(pointer[s] == v) is synthesized on-chip (iota + is_equal, bf16 holds 0/1
exactly) and contracted against bf16 alpha. The scatter is sparse: only ~65
of the 512 source positions point into any given macro-tile, so the host
groups source indices by macro-tile (index metadata only), the device
gathers the <=128 relevant alpha rows per macro-tile with an indirect DMA,
and the contraction runs with K=128 instead of K=512. The PSUM drain applies
the (1-p_gen) scale on the scalar engine; the vector engine fuses
p_gen*dist with an all-16-bit scalar_tensor_tensor. If any macro-tile owns
more than 128 pointers (probability ~1e-9 for uniform pointers), a dense
f32 K=512 fallback makes no assumption about pointer distribution.
"""
import sys

sys.path.insert(0, "/opt/trn_rl_repo")

import numpy as np
import ml_dtypes

import concourse.bacc as bacc
import concourse.bass as bass
import concourse.tile as tile
from concourse import mybir
from concourse.bass_utils import run_bass_kernel_spmd

B = 8
L_DEC = 256
V = 32000
L_SRC = 512
V_EXT = 32128
P = 128
NCORES = 8
NPSUM = 512   # psum tile width (one PSUM bank at fp32)

F32 = mybir.dt.float32
BF16 = mybir.dt.bfloat16
I16 = mybir.dt.int16
I32 = mybir.dt.int32

MACRO_SPARSE = 4096
N_MACRO_SPARSE = (V_EXT + MACRO_SPARSE - 1) // MACRO_SPARSE  # 8 (last 3456)

_NC_CACHE = {}


def _build_nc_sparse():
    """K=128-per-macro-tile variant: host-grouped pointers, device gathers.

    All bulk I/O in bf16. Engine roles are fully decoupled so the DMA
    streams never wait on a busy engine:
      sync   - dist loads only (own HWDGE ring)
      scalar - iota/meta prologue + output stores (own HWDGE ring)
      gpsimd - alpha gathers + (1-p_gen) fold (SWDGE + broadcast)
      tensor - one-hot matmuls (q pre-folded into alpha)
      vector - one-hot build + single fused drain pass:
               out = p_gen*dist + psum   (psum already holds q*copyp)
    """
    nc = bacc.Bacc("TRN2", target_bir_lowering=False, debug=False)
    dist_d = nc.dram_tensor("dist", [L_DEC, V], BF16, kind="ExternalInput").ap()
    pgen_d = nc.dram_tensor("pgen", [L_DEC, 1], F32, kind="ExternalInput").ap()
    alpha_d = nc.dram_tensor("alpha", [L_SRC, L_DEC], BF16, kind="ExternalInput").ap()
    out_d = nc.dram_tensor("out", [L_DEC, V_EXT], BF16, kind="ExternalOutput").ap()
    # per macro-tile metadata, pre-transposed on host to a DMA-friendly
    # [128, 16] layout: col m = source row indices (padded with 0),
    # col N_MACRO+m = pointer values (padded with -1)
    meta_d = nc.dram_tensor(
        "meta", [P, 2 * N_MACRO_SPARSE], I32, kind="ExternalInput"
    ).ap()
    iota_d = nc.dram_tensor(
        "iota", [P, MACRO_SPARSE], I16, kind="ExternalInput"
    ).ap()

    n_tchunk = L_DEC // P
    MACRO = MACRO_SPARSE
    CHUNK = 2048  # vector-drain granularity: one 4-bank PSUM tile

    with tile.TileContext(nc) as tc:
        with (
            tc.tile_pool(name="const", bufs=1) as cpool,
            tc.tile_pool(name="dist", bufs=5) as dpool,
            tc.tile_pool(name="outp", bufs=5) as opool,
            tc.tile_pool(name="oh", bufs=8) as ohpool,
            tc.tile_pool(name="psum", bufs=2, space="PSUM") as pspool,
        ):
            # ---- prologue ----
            meta_sb = cpool.tile([P, 2 * N_MACRO_SPARSE], I32)
            nc.scalar.dma_start(meta_sb[:], meta_d[:])
            pgen_sb = cpool.tile([P, n_tchunk], F32)
            for t in range(n_tchunk):
                nc.sync.dma_start(
                    pgen_sb[:, t : t + 1], pgen_d[t * P : (t + 1) * P, 0:1]
                )
            # q as a [1, L_DEC] row, broadcast to all partitions for the
            # alpha fold (ag_q[s, t] = ag[s, t] * (1 - p_gen[t]))
            pgen_row = cpool.tile([1, L_DEC], F32)
            nc.scalar.dma_start(
                pgen_row[:], pgen_d.rearrange("t one -> one t")
            )
            iota16 = cpool.tile([P, MACRO], I16)
            nc.scalar.dma_start(iota16[:], iota_d[:])

            q_row = cpool.tile([1, L_DEC], F32)
            nc.vector.tensor_scalar(
                out=q_row[:], in0=pgen_row[:], scalar1=-1.0, scalar2=1.0,
                op0=mybir.AluOpType.mult, op1=mybir.AluOpType.add,
            )
            q_bcast = cpool.tile([P, L_DEC], F32)
            nc.gpsimd.partition_broadcast(q_bcast[:], q_row[0:1, :])

            # alpha-row gathers: one SWDGE op type only (no Q7 library
            # thrash); the q fold runs on the vector engine
            ag_all = []
            for m in range(N_MACRO_SPARSE):
                ag = cpool.tile([P, L_DEC], BF16, tag=f"ag{m}")
                nc.gpsimd.indirect_dma_start(
                    out=ag[:],
                    out_offset=None,
                    in_=alpha_d[:],
                    in_offset=bass.IndirectOffsetOnAxis(
                        ap=meta_sb[:, m : m + 1], axis=0
                    ),
                )
                ag_all.append(ag)

            # all per-macro shifts up-front (tiny)
            shift_all = cpool.tile([P, N_MACRO_SPARSE], F32)
            for m in range(N_MACRO_SPARSE):
                nc.vector.tensor_scalar(
                    out=shift_all[:, m : m + 1],
                    in0=meta_sb[:, N_MACRO_SPARSE + m : N_MACRO_SPARSE + m + 1],
                    scalar1=float(m * MACRO),
                    scalar2=None, op0=mybir.AluOpType.subtract,
                )

            # q fold + all one-hots built up-front on the vector engine, so
            # the steady-state stt stream never waits on oh/agq deps
            agq_all = []
            oh_all = []
            for m in range(N_MACRO_SPARSE):
                vw = min(MACRO, V_EXT - m * MACRO)
                agq = cpool.tile([P, L_DEC], BF16, tag=f"agq{m}")
                nc.vector.tensor_tensor(
                    out=agq[:], in0=ag_all[m][:], in1=q_bcast[:],
                    op=mybir.AluOpType.mult,
                )
                agq_all.append(agq)
                oh = ohpool.tile([P, MACRO], BF16, tag="oh")
                nc.vector.tensor_scalar(
                    out=oh[:, :vw], in0=iota16[:, :vw],
                    scalar1=shift_all[:, m : m + 1], scalar2=None,
                    op0=mybir.AluOpType.is_equal,
                )
                oh_all.append(oh)

            # ---- main loop ----
            for m in range(N_MACRO_SPARSE):
                v0 = m * MACRO
                vw = min(MACRO, V_EXT - v0)
                dw = max(0, min(vw, V - v0))
                agq = agq_all[m]
                oh = oh_all[m]

                for t in range(n_tchunk):
                    trow = slice(t * P, (t + 1) * P)
                    dist_sb = dpool.tile([P, MACRO], BF16, tag="dist")
                    if dw > 0:
                        nc.sync.dma_start(dist_sb[:, :dw], dist_d[trow, v0 : v0 + dw])
                    out_sb = opool.tile([P, MACRO], BF16, tag="out")

                    for c0 in range(0, vw, CHUNK):
                        cw = min(CHUNK, vw - c0)
                        psum = pspool.tile([P, CHUNK], F32, space="PSUM")
                        for j0 in range(0, cw, NPSUM):
                            jw = min(NPSUM, cw - j0)
                            nc.tensor.matmul(
                                out=psum[:, j0 : j0 + jw],
                                lhsT=agq[:, trow],
                                rhs=oh[:, c0 + j0 : c0 + j0 + jw],
                                start=True, stop=True,
                            )
                        # fused drain: out = p_gen*dist + q*copyp (one pass)
                        fw = min(dw, c0 + cw) - c0
                        if fw > 0:
                            nc.vector.scalar_tensor_tensor(
                                out=out_sb[:, c0 : c0 + fw],
                                in0=dist_sb[:, c0 : c0 + fw],
                                scalar=pgen_sb[:, t : t + 1],
                                in1=psum[:, :fw],
                                op0=mybir.AluOpType.mult, op1=mybir.AluOpType.add,
                            )
                        if fw < cw:  # pad columns beyond V: out = q*copyp
                            pw = cw - max(fw, 0)
                            nc.scalar.activation(
                                out=out_sb[:, c0 + cw - pw : c0 + cw],
                                in_=psum[:, cw - pw : cw],
                                func=mybir.ActivationFunctionType.Copy,
                            )
                        # store per chunk: smooth write stream, early start
                        nc.scalar.dma_start(
                            out_d[trow, v0 + c0 : v0 + c0 + cw],
                            out_sb[:, c0 : c0 + cw],
                        )
    nc.compile()
    return nc


def _build_nc_dense():
    """Dense f32 K=512 fallback: no assumption on pointer distribution."""
    MACRO = 2048
    nc = bacc.Bacc("TRN2", target_bir_lowering=False, debug=False)
    dist_d = nc.dram_tensor("dist", [L_DEC, V], F32, kind="ExternalInput").ap()
    pgen_d = nc.dram_tensor("pgen", [L_DEC, 1], F32, kind="ExternalInput").ap()
    alpha_d = nc.dram_tensor("alpha", [L_SRC, L_DEC], F32, kind="ExternalInput").ap()
    out_d = nc.dram_tensor("out", [L_DEC, V_EXT], F32, kind="ExternalOutput").ap()
    ptr_d = nc.dram_tensor("ptr", [L_SRC, 1], I32, kind="ExternalInput").ap()

    n_schunk = L_SRC // P
    n_tchunk = L_DEC // P
    n_macro = (V_EXT + MACRO - 1) // MACRO

    with tile.TileContext(nc) as tc:
        with (
            tc.tile_pool(name="const", bufs=1) as cpool,
            tc.tile_pool(name="dist", bufs=3) as dpool,
            tc.tile_pool(name="outp", bufs=3) as opool,
            tc.tile_pool(name="oh", bufs=2) as ohpool,
            tc.tile_pool(name="psum", bufs=6, space="PSUM") as pspool,
        ):
            ptr_sb = cpool.tile([P, n_schunk], I32)
            for c in range(n_schunk):
                nc.sync.dma_start(ptr_sb[:, c : c + 1], ptr_d[c * P : (c + 1) * P, 0:1])
            pgen_sb = cpool.tile([P, n_tchunk], F32)
            for t in range(n_tchunk):
                nc.sync.dma_start(
                    pgen_sb[:, t : t + 1], pgen_d[t * P : (t + 1) * P, 0:1]
                )
            q_sb = cpool.tile([P, n_tchunk], F32)
            nc.vector.tensor_scalar(
                out=q_sb[:], in0=pgen_sb[:], scalar1=-1.0, scalar2=1.0,
                op0=mybir.AluOpType.mult, op1=mybir.AluOpType.add,
            )
            alpha_terms = []  # per chunk: (hi, mid, lo) bf16
            for c in range(n_schunk):
                a = cpool.tile([P, L_DEC], F32, tag=f"alpha{c}")
                nc.sync.dma_start(a[:], alpha_d[c * P : (c + 1) * P, :])
                hi = cpool.tile([P, L_DEC], BF16, tag=f"ahi{c}")
                nc.vector.tensor_copy(hi[:], a[:])
                r1 = cpool.tile([P, L_DEC], F32, tag=f"r1{c}")
                nc.vector.tensor_tensor(
                    out=r1[:], in0=a[:], in1=hi[:], op=mybir.AluOpType.subtract
                )
                mid = cpool.tile([P, L_DEC], BF16, tag=f"amid{c}")
                nc.vector.tensor_copy(mid[:], r1[:])
                lo = cpool.tile([P, L_DEC], BF16, tag=f"alo{c}")
                nc.vector.tensor_tensor(
                    out=lo[:], in0=r1[:], in1=mid[:], op=mybir.AluOpType.subtract
                )
                alpha_terms.append((hi, mid, lo))
            iota16 = cpool.tile([P, MACRO], I16)
            nc.gpsimd.iota(iota16[:], pattern=[[1, MACRO]], base=0, channel_multiplier=0)

            for m in range(n_macro):
                v0 = m * MACRO
                vw = min(MACRO, V_EXT - v0)
                dw = max(0, min(vw, V - v0))
                shift = ohpool.tile([P, n_schunk], F32, tag="shift")
                nc.vector.tensor_scalar(
                    out=shift[:], in0=ptr_sb[:], scalar1=float(v0), scalar2=None,
                    op0=mybir.AluOpType.subtract,
                )
                ohs = []
                for c in range(n_schunk):
                    oh = ohpool.tile([P, MACRO], BF16, tag=f"oh{c}")
                    nc.vector.tensor_scalar(
                        out=oh[:, :vw], in0=iota16[:, :vw],
                        scalar1=shift[:, c : c + 1], scalar2=None,
                        op0=mybir.AluOpType.is_equal,
                    )
                    ohs.append(oh)
                for t in range(n_tchunk):
                    trow = slice(t * P, (t + 1) * P)
                    dist_sb = dpool.tile([P, MACRO], F32, tag="dist")
                    if dw > 0:
                        nc.sync.dma_start(dist_sb[:, :dw], dist_d[trow, v0 : v0 + dw])
                    out_sb = opool.tile([P, MACRO], F32, tag="out")
                    nj = (vw + NPSUM - 1) // NPSUM
                    for j in range(nj):
                        jw = min(NPSUM, vw - j * NPSUM)
                        psum = pspool.tile([P, NPSUM], F32, space="PSUM")
                        mm_list = [
                            (c, amat)
                            for term in range(3)
                            for c in range(n_schunk)
                            for amat in (alpha_terms[c][term],)
                        ]
                        for k, (c, amat) in enumerate(mm_list):
                            nc.tensor.matmul(
                                out=psum[:, :jw],
                                lhsT=amat[:, trow],
                                rhs=ohs[c][:, j * NPSUM : j * NPSUM + jw],
                                start=(k == 0), stop=(k == len(mm_list) - 1),
                            )
                        nc.scalar.activation(
                            out=out_sb[:, j * NPSUM : j * NPSUM + jw],
                            in_=psum[:, :jw],
                            func=mybir.ActivationFunctionType.Copy,
                            scale=q_sb[:, t : t + 1],
                        )
                    if dw > 0:
                        nc.vector.scalar_tensor_tensor(
                            out=out_sb[:, :dw], in0=dist_sb[:, :dw],
                            scalar=pgen_sb[:, t : t + 1], in1=out_sb[:, :dw],
                            op0=mybir.AluOpType.mult, op1=mybir.AluOpType.add,
                        )
                    nc.sync.dma_start(out_d[trow, v0 : v0 + vw], out_sb[:, :vw])
    nc.compile()
    return nc


def _get_nc(variant):
    if variant not in _NC_CACHE:
        _NC_CACHE[variant] = (
            _build_nc_sparse() if variant == "sparse" else _build_nc_dense()
        )
    return _NC_CACHE[variant]


_IOTA = None


def _iota_const():
    global _IOTA
    if _IOTA is None:
        _IOTA = np.ascontiguousarray(
            np.broadcast_to(
                np.arange(MACRO_SPARSE, dtype=np.int16), (P, MACRO_SPARSE)
            )
        )
    return _IOTA


def _group_pointers(ptr_b):
    """Group source indices by owning macro-tile. Returns meta [P, 2*N_MACRO]
    int32 (col m: source row indices padded with 0; col N_MACRO+m: pointer
    values padded with -1), or None if any tile owns > P pointers."""
    owner = ptr_b // MACRO_SPARSE
    meta = np.zeros((P, 2 * N_MACRO_SPARSE), np.int32)
    meta[:, N_MACRO_SPARSE:] = -1
    for m in range(N_MACRO_SPARSE):
        sel = np.nonzero(owner == m)[0]
        if len(sel) > P:
            return None
        meta[: len(sel), m] = sel
        meta[: len(sel), N_MACRO_SPARSE + m] = ptr_b[sel]
    return meta


def _prep(dist_t, p_gen, alph_t, pointer):
    dist_t = np.ascontiguousarray(np.asarray(dist_t, dtype=np.float32))
    p_gen = np.ascontiguousarray(
        np.asarray(p_gen, dtype=np.float32).reshape(B, L_DEC, 1)
    )
    alph_t = np.ascontiguousarray(np.asarray(alph_t, dtype=np.float32))
    ptr = np.asarray(pointer).astype(np.int32).reshape(B, L_SRC)
    assert dist_t.shape == (B, L_DEC, V), dist_t.shape
    assert alph_t.shape == (B, L_SRC, L_DEC), alph_t.shape

    in_maps = []
    variant = "sparse"
    metas = []
    for b in range(B):
        meta = _group_pointers(ptr[b])
        if meta is None:
            variant = "dense"
            break
        metas.append(np.ascontiguousarray(meta))
    if variant == "sparse":
        dist_bf = dist_t.astype(ml_dtypes.bfloat16)
        alph_bf = alph_t.astype(ml_dtypes.bfloat16)
        in_maps = [
            {"dist": dist_bf[b], "pgen": p_gen[b], "alpha": alph_bf[b],
             "meta": metas[b], "iota": _iota_const()}
            for b in range(B)
        ]
    else:
        in_maps = [
            {"dist": dist_t[b], "pgen": p_gen[b], "alpha": alph_t[b],
             "ptr": np.ascontiguousarray(ptr[b].reshape(L_SRC, 1))}
            for b in range(B)
        ]
    return variant, in_maps


def run(dist_t, p_gen, alph_t, batch_vocab, pointer, trace=False,
        force_variant=None, **spmd_kwargs):
    """Run the kernel; returns (output, BassKernelResults)."""
    assert batch_vocab.shape[0] == V_EXT
    variant, in_maps = _prep(dist_t, p_gen, alph_t, pointer)
    if force_variant == "dense" and variant == "sparse":
        ptr = np.asarray(pointer).astype(np.int32).reshape(B, L_SRC)
        dist_f = np.ascontiguousarray(np.asarray(dist_t, dtype=np.float32))
        alph_f = np.ascontiguousarray(np.asarray(alph_t, dtype=np.float32))
        for b in range(B):
            m = in_maps[b]
            del m["meta"]
            m["dist"] = dist_f[b]
            m["alpha"] = alph_f[b]
            m["ptr"] = np.ascontiguousarray(ptr[b].reshape(L_SRC, 1))
        variant = "dense"
    run.last_variant = variant
    res = None
    for attempt in range(3):
        try:
            res = run_bass_kernel_spmd(
                _get_nc(variant), in_maps, list(range(NCORES)),
                trace=trace and attempt == 0, **spmd_kwargs
            )
            break
        except Exception:
            # transient device-state failures (e.g. NRT_EXEC_UNIT_UNRECOVERABLE
            # left over from a previous profiled session) sometimes clear on
            # retry; give it two more chances (untraced -- profiling itself
            # can be the destabilizer) before giving up
            if attempt == 2:
                raise
            import time

            time.sleep(2.0)
    out = np.stack(
        [np.asarray(res.results[b]["out"]).astype(np.float32) for b in range(B)],
        axis=0,
    )
    return out, res


def kernel(dist_t, p_gen, alph_t, batch_vocab, pointer):
    out, _ = run(dist_t, p_gen, alph_t, batch_vocab, pointer)
    return out


# revision 14
# speedup vs baseline: 1.2005x; 1.2005x over previous
"""CopyNet extended-vocab projection kernel for Trainium2 (8 NeuronCores).

out[b, t, v] = p_gen[b,t] * pad(dist_t)[b,t,v] + (1 - p_gen[b,t]) * copyp[b,t,v]
copyp[b, t, v] = sum_{s: pointer[b,s]==v} alph_t[b, s, t]

Strategy: pure data-parallel over batch (B=8 -> 8 cores, one batch element per
core). The kernel is HBM-bound, so all bulk streams are bf16 (the rel-err
budget is 2e-2; the full bf16 chain is ~6e-3 worst-case): the host ships dist
and alpha as bf16, the device writes a bf16 output, and the host upcasts to
f32. Per-core HBM traffic is 16.4 MB (dist in) + 16.5 MB (out) + 0.3 MB
(alpha) ~= 33 MB vs 66 MB for the all-f32 variant.

Per core the output streams through SBUF in 4096-wide vocab macro-tiles; the
copy term is a one-hot matmul on the tensor engine: onehot[s, v] =
(pointer[s] == v) is synthesized on-chip (iota + is_equal, bf16 holds 0/1
exactly) and contracted against bf16 alpha. The scatter is sparse: only ~65
of the 512 source positions point into any given macro-tile, so the host
groups source indices by macro-tile (index metadata only), the device
gathers the <=128 relevant alpha rows per macro-tile with an indirect DMA,
and the contraction runs with K=128 instead of K=512. The PSUM drain applies
the (1-p_gen) scale on the scalar engine; the vector engine fuses
p_gen*dist with an all-16-bit scalar_tensor_tensor. If any macro-tile owns
more than 128 pointers (probability ~1e-9 for uniform pointers), a dense
f32 K=512 fallback makes no assumption about pointer distribution.
"""
import sys

sys.path.insert(0, "/opt/trn_rl_repo")

import numpy as np
import ml_dtypes

import concourse.bacc as bacc
import concourse.bass as bass
import concourse.tile as tile
from concourse import mybir
from concourse.bass_utils import run_bass_kernel_spmd

B = 8
L_DEC = 256
V = 32000
L_SRC = 512
V_EXT = 32128
P = 128
NCORES = 8
NPSUM = 512   # psum tile width (one PSUM bank at fp32)

F32 = mybir.dt.float32
BF16 = mybir.dt.bfloat16
I16 = mybir.dt.int16
I32 = mybir.dt.int32

MACRO_SPARSE = 4096
N_MACRO_SPARSE = (V_EXT + MACRO_SPARSE - 1) // MACRO_SPARSE  # 8 (last 3456)

_NC_CACHE = {}


def _build_nc_sparse():
    """K=128-per-macro-tile variant: host-grouped pointers, device gathers.

    All bulk I/O in bf16. Engine roles are fully decoupled so the DMA
    streams never wait on a busy engine:
      sync   - dist loads only (own HWDGE ring)
      scalar - iota/meta prologue + output stores (own HWDGE ring)
      gpsimd - alpha gathers + (1-p_gen) fold (SWDGE + broadcast)
      tensor - one-hot matmuls (q pre-folded into alpha)
      vector - one-hot build + single fused drain pass:
               out = p_gen*dist + psum   (psum already holds q*copyp)
    """
    nc = bacc.Bacc("TRN2", target_bir_lowering=False, debug=False)
    dist_d = nc.dram_tensor("dist", [L_DEC, V], BF16, kind="ExternalInput").ap()
    pgen_d = nc.dram_tensor("pgen", [L_DEC, 1], F32, kind="ExternalInput").ap()
    alpha_d = nc.dram_tensor("alpha", [L_SRC, L_DEC], BF16, kind="ExternalInput").ap()
    out_d = nc.dram_tensor("out", [L_DEC, V_EXT], BF16, kind="ExternalOutput").ap()
    # per macro-tile metadata, pre-transposed on host to a DMA-friendly
    # [128, 16] layout: col m = source row indices (padded with 0),
    # col N_MACRO+m = pointer values (padded with -1)
    meta_d = nc.dram_tensor(
        "meta", [P, 2 * N_MACRO_SPARSE], I32, kind="ExternalInput"
    ).ap()

    n_tchunk = L_DEC // P
    MACRO = MACRO_SPARSE
    CHUNK = 2048  # vector-drain granularity: one 4-bank PSUM tile

    with tile.TileContext(nc) as tc:
        with (
            tc.tile_pool(name="const", bufs=1) as cpool,
            tc.tile_pool(name="dist", bufs=6) as dpool,
            tc.tile_pool(name="outp", bufs=5) as opool,
            tc.tile_pool(name="oh", bufs=4) as ohpool,
            tc.tile_pool(name="psum", bufs=2, space="PSUM") as pspool,
        ):
            # ---- prologue ----
            meta_sb = cpool.tile([P, 2 * N_MACRO_SPARSE], I32)
            nc.scalar.dma_start(meta_sb[:], meta_d[:])
            pgen_sb = cpool.tile([P, n_tchunk], F32)
            for t in range(n_tchunk):
                nc.sync.dma_start(
                    pgen_sb[:, t : t + 1], pgen_d[t * P : (t + 1) * P, 0:1]
                )
            pgen_row = cpool.tile([1, L_DEC], F32)
            nc.scalar.dma_start(
                pgen_row[:], pgen_d.rearrange("t one -> one t")
            )
            # on-chip iota (host ship / partition_broadcast both proved slow)
            iota16 = cpool.tile([P, MACRO], I16)
            nc.gpsimd.iota(
                iota16[:], pattern=[[1, MACRO]], base=0, channel_multiplier=0
            )

            # q broadcast [128, L_DEC] via a K=1 outer product on the (idle)
            # tensor engine: ones[1,128]^T @ q_row[1,256]
            q_row_bf = cpool.tile([1, L_DEC], BF16)
            nc.vector.tensor_scalar(
                out=q_row_bf[:], in0=pgen_row[:], scalar1=-1.0, scalar2=1.0,
                op0=mybir.AluOpType.mult, op1=mybir.AluOpType.add,
            )
            ones_bf = cpool.tile([1, P], BF16)
            nc.vector.tensor_scalar(
                out=ones_bf[:], in0=pgen_row[0:1, 0:P], scalar1=0.0, scalar2=1.0,
                op0=mybir.AluOpType.mult, op1=mybir.AluOpType.add,
            )
            psum_q = pspool.tile([P, CHUNK], F32, space="PSUM", tag="ps")
            nc.tensor.matmul(
                out=psum_q[:, :L_DEC], lhsT=ones_bf[:], rhs=q_row_bf[:],
                start=True, stop=True,
            )
            q_bcast = cpool.tile([P, L_DEC], F32)
            nc.vector.tensor_copy(q_bcast[:], psum_q[:, :L_DEC])

            # alpha-row gathers (SWDGE, one op type -> one Q7 library load)
            ag_all = []
            for m in range(N_MACRO_SPARSE):
                ag = cpool.tile([P, L_DEC], BF16, tag=f"ag{m}")
                nc.gpsimd.indirect_dma_start(
                    out=ag[:],
                    out_offset=None,
                    in_=alpha_d[:],
                    in_offset=bass.IndirectOffsetOnAxis(
                        ap=meta_sb[:, m : m + 1], axis=0
                    ),
                )
                ag_all.append(ag)

            # all per-macro shifts up-front (tiny)
            shift_all = cpool.tile([P, N_MACRO_SPARSE], F32)
            for m in range(N_MACRO_SPARSE):
                nc.vector.tensor_scalar(
                    out=shift_all[:, m : m + 1],
                    in0=meta_sb[:, N_MACRO_SPARSE + m : N_MACRO_SPARSE + m + 1],
                    scalar1=float(m * MACRO),
                    scalar2=None, op0=mybir.AluOpType.subtract,
                )

            # q fold + one-hot build for macro m (vector engine)
            agq_all = {}

            def build_macro(m):
                vw_ = min(MACRO, V_EXT - m * MACRO)
                agq = cpool.tile([P, L_DEC], BF16, tag=f"agq{m}")
                nc.vector.tensor_tensor(
                    out=agq[:], in0=ag_all[m][:], in1=q_bcast[:],
                    op=mybir.AluOpType.mult,
                )
                oh = ohpool.tile([P, MACRO], BF16, tag="oh")
                nc.vector.tensor_scalar(
                    out=oh[:, :vw_], in0=iota16[:, :vw_],
                    scalar1=shift_all[:, m : m + 1], scalar2=None,
                    op0=mybir.AluOpType.is_equal,
                )
                agq_all[m] = (agq, oh)

            build_macro(0)
            build_macro(1)

            # ---- main loop (macro m+2 built between macro stt batches) ----
            for m in range(N_MACRO_SPARSE):
                v0 = m * MACRO
                vw = min(MACRO, V_EXT - v0)
                dw = max(0, min(vw, V - v0))
                agq, oh = agq_all[m]

                for t in range(n_tchunk):
                    if t == 1 and m + 2 < N_MACRO_SPARSE:
                        build_macro(m + 2)
                    trow = slice(t * P, (t + 1) * P)
                    dist_sb = dpool.tile([P, MACRO], BF16, tag="dist")
                    if dw > 0:
                        nc.sync.dma_start(dist_sb[:, :dw], dist_d[trow, v0 : v0 + dw])
                    out_sb = opool.tile([P, MACRO], BF16, tag="out")

                    for c0 in range(0, vw, CHUNK):
                        cw = min(CHUNK, vw - c0)
                        psum = pspool.tile([P, CHUNK], F32, space="PSUM", tag="ps")
                        for j0 in range(0, cw, NPSUM):
                            jw = min(NPSUM, cw - j0)
                            nc.tensor.matmul(
                                out=psum[:, j0 : j0 + jw],
                                lhsT=agq[:, trow],
                                rhs=oh[:, c0 + j0 : c0 + j0 + jw],
                                start=True, stop=True,
                            )
                        # fused drain: out = p_gen*dist + q*copyp (one pass)
                        fw = min(dw, c0 + cw) - c0
                        if fw > 0:
                            nc.vector.scalar_tensor_tensor(
                                out=out_sb[:, c0 : c0 + fw],
                                in0=dist_sb[:, c0 : c0 + fw],
                                scalar=pgen_sb[:, t : t + 1],
                                in1=psum[:, :fw],
                                op0=mybir.AluOpType.mult, op1=mybir.AluOpType.add,
                            )
                        if fw < cw:  # pad columns beyond V: out = q*copyp
                            pw = cw - max(fw, 0)
                            nc.scalar.activation(
                                out=out_sb[:, c0 + cw - pw : c0 + cw],
                                in_=psum[:, cw - pw : cw],
                                func=mybir.ActivationFunctionType.Copy,
                            )
                        # store per chunk: smooth write stream, early start
                        nc.scalar.dma_start(
                            out_d[trow, v0 + c0 : v0 + c0 + cw],
                            out_sb[:, c0 : c0 + cw],
                        )
    nc.compile()
    return nc


def _build_nc_dense():
    """Dense f32 K=512 fallback: no assumption on pointer distribution."""
    MACRO = 2048
    nc = bacc.Bacc("TRN2", target_bir_lowering=False, debug=False)
    dist_d = nc.dram_tensor("dist", [L_DEC, V], F32, kind="ExternalInput").ap()
    pgen_d = nc.dram_tensor("pgen", [L_DEC, 1], F32, kind="ExternalInput").ap()
    alpha_d = nc.dram_tensor("alpha", [L_SRC, L_DEC], F32, kind="ExternalInput").ap()
    out_d = nc.dram_tensor("out", [L_DEC, V_EXT], F32, kind="ExternalOutput").ap()
    ptr_d = nc.dram_tensor("ptr", [L_SRC, 1], I32, kind="ExternalInput").ap()

    n_schunk = L_SRC // P
    n_tchunk = L_DEC // P
    n_macro = (V_EXT + MACRO - 1) // MACRO

    with tile.TileContext(nc) as tc:
        with (
            tc.tile_pool(name="const", bufs=1) as cpool,
            tc.tile_pool(name="dist", bufs=3) as dpool,
            tc.tile_pool(name="outp", bufs=3) as opool,
            tc.tile_pool(name="oh", bufs=2) as ohpool,
            tc.tile_pool(name="psum", bufs=6, space="PSUM") as pspool,
        ):
            ptr_sb = cpool.tile([P, n_schunk], I32)
            for c in range(n_schunk):
                nc.sync.dma_start(ptr_sb[:, c : c + 1], ptr_d[c * P : (c + 1) * P, 0:1])
            pgen_sb = cpool.tile([P, n_tchunk], F32)
            for t in range(n_tchunk):
                nc.sync.dma_start(
                    pgen_sb[:, t : t + 1], pgen_d[t * P : (t + 1) * P, 0:1]
                )
            q_sb = cpool.tile([P, n_tchunk], F32)
            nc.vector.tensor_scalar(
                out=q_sb[:], in0=pgen_sb[:], scalar1=-1.0, scalar2=1.0,
                op0=mybir.AluOpType.mult, op1=mybir.AluOpType.add,
            )
            alpha_terms = []  # per chunk: (hi, mid, lo) bf16
            for c in range(n_schunk):
                a = cpool.tile([P, L_DEC], F32, tag=f"alpha{c}")
                nc.sync.dma_start(a[:], alpha_d[c * P : (c + 1) * P, :])
                hi = cpool.tile([P, L_DEC], BF16, tag=f"ahi{c}")
                nc.vector.tensor_copy(hi[:], a[:])
                r1 = cpool.tile([P, L_DEC], F32, tag=f"r1{c}")
                nc.vector.tensor_tensor(
                    out=r1[:], in0=a[:], in1=hi[:], op=mybir.AluOpType.subtract
                )
                mid = cpool.tile([P, L_DEC], BF16, tag=f"amid{c}")
                nc.vector.tensor_copy(mid[:], r1[:])
                lo = cpool.tile([P, L_DEC], BF16, tag=f"alo{c}")
                nc.vector.tensor_tensor(
                    out=lo[:], in0=r1[:], in1=mid[:], op=mybir.AluOpType.subtract
                )
                alpha_terms.append((hi, mid, lo))
            iota16 = cpool.tile([P, MACRO], I16)
            nc.gpsimd.iota(iota16[:], pattern=[[1, MACRO]], base=0, channel_multiplier=0)

            for m in range(n_macro):
                v0 = m * MACRO
                vw = min(MACRO, V_EXT - v0)
                dw = max(0, min(vw, V - v0))
                shift = ohpool.tile([P, n_schunk], F32, tag="shift")
                nc.vector.tensor_scalar(
                    out=shift[:], in0=ptr_sb[:], scalar1=float(v0), scalar2=None,
                    op0=mybir.AluOpType.subtract,
                )
                ohs = []
                for c in range(n_schunk):
                    oh = ohpool.tile([P, MACRO], BF16, tag=f"oh{c}")
                    nc.vector.tensor_scalar(
                        out=oh[:, :vw], in0=iota16[:, :vw],
                        scalar1=shift[:, c : c + 1], scalar2=None,
                        op0=mybir.AluOpType.is_equal,
                    )
                    ohs.append(oh)
                for t in range(n_tchunk):
                    trow = slice(t * P, (t + 1) * P)
                    dist_sb = dpool.tile([P, MACRO], F32, tag="dist")
                    if dw > 0:
                        nc.sync.dma_start(dist_sb[:, :dw], dist_d[trow, v0 : v0 + dw])
                    out_sb = opool.tile([P, MACRO], F32, tag="out")
                    nj = (vw + NPSUM - 1) // NPSUM
                    for j in range(nj):
                        jw = min(NPSUM, vw - j * NPSUM)
                        psum = pspool.tile([P, NPSUM], F32, space="PSUM")
                        mm_list = [
                            (c, amat)
                            for term in range(3)
                            for c in range(n_schunk)
                            for amat in (alpha_terms[c][term],)
                        ]
                        for k, (c, amat) in enumerate(mm_list):
                            nc.tensor.matmul(
                                out=psum[:, :jw],
                                lhsT=amat[:, trow],
                                rhs=ohs[c][:, j * NPSUM : j * NPSUM + jw],
                                start=(k == 0), stop=(k == len(mm_list) - 1),
                            )
                        nc.scalar.activation(
                            out=out_sb[:, j * NPSUM : j * NPSUM + jw],
                            in_=psum[:, :jw],
                            func=mybir.ActivationFunctionType.Copy,
                            scale=q_sb[:, t : t + 1],
                        )
                    if dw > 0:
                        nc.vector.scalar_tensor_tensor(
                            out=out_sb[:, :dw], in0=dist_sb[:, :dw],
                            scalar=pgen_sb[:, t : t + 1], in1=out_sb[:, :dw],
                            op0=mybir.AluOpType.mult, op1=mybir.AluOpType.add,
                        )
                    nc.sync.dma_start(out_d[trow, v0 : v0 + vw], out_sb[:, :vw])
    nc.compile()
    return nc


def _get_nc(variant):
    if variant not in _NC_CACHE:
        _NC_CACHE[variant] = (
            _build_nc_sparse() if variant == "sparse" else _build_nc_dense()
        )
    return _NC_CACHE[variant]


_IOTA = None


def _iota_const():
    global _IOTA
    if _IOTA is None:
        _IOTA = np.ascontiguousarray(
            np.broadcast_to(
                np.arange(MACRO_SPARSE, dtype=np.int16), (P, MACRO_SPARSE)
            )
        )
    return _IOTA


def _group_pointers(ptr_b):
    """Group source indices by owning macro-tile. Returns meta [P, 2*N_MACRO]
    int32 (col m: source row indices padded with 0; col N_MACRO+m: pointer
    values padded with -1), or None if any tile owns > P pointers."""
    owner = ptr_b // MACRO_SPARSE
    meta = np.zeros((P, 2 * N_MACRO_SPARSE), np.int32)
    meta[:, N_MACRO_SPARSE:] = -1
    for m in range(N_MACRO_SPARSE):
        sel = np.nonzero(owner == m)[0]
        if len(sel) > P:
            return None
        meta[: len(sel), m] = sel
        meta[: len(sel), N_MACRO_SPARSE + m] = ptr_b[sel]
    return meta


def _prep(dist_t, p_gen, alph_t, pointer):
    dist_t = np.ascontiguousarray(np.asarray(dist_t, dtype=np.float32))
    p_gen = np.ascontiguousarray(
        np.asarray(p_gen, dtype=np.float32).reshape(B, L_DEC, 1)
    )
    alph_t = np.ascontiguousarray(np.asarray(alph_t, dtype=np.float32))
    ptr = np.asarray(pointer).astype(np.int32).reshape(B, L_SRC)
    assert dist_t.shape == (B, L_DEC, V), dist_t.shape
    assert alph_t.shape == (B, L_SRC, L_DEC), alph_t.shape

    in_maps = []
    variant = "sparse"
    metas = []
    for b in range(B):
        meta = _group_pointers(ptr[b])
        if meta is None:
            variant = "dense"
            break
        metas.append(np.ascontiguousarray(meta))
    if variant == "sparse":
        dist_bf = dist_t.astype(ml_dtypes.bfloat16)
        alph_bf = alph_t.astype(ml_dtypes.bfloat16)
        in_maps = [
            {"dist": dist_bf[b], "pgen": p_gen[b], "alpha": alph_bf[b],
             "meta": metas[b]}
            for b in range(B)
        ]
    else:
        in_maps = [
            {"dist": dist_t[b], "pgen": p_gen[b], "alpha": alph_t[b],
             "ptr": np.ascontiguousarray(ptr[b].reshape(L_SRC, 1))}
            for b in range(B)
        ]
    return variant, in_maps


def run(dist_t, p_gen, alph_t, batch_vocab, pointer, trace=False,
        force_variant=None, **spmd_kwargs):
    """Run the kernel; returns (output, BassKernelResults)."""
    assert batch_vocab.shape[0] == V_EXT
    variant, in_maps = _prep(dist_t, p_gen, alph_t, pointer)
    if force_variant == "dense" and variant == "sparse":
        ptr = np.asarray(pointer).astype(np.int32).reshape(B, L_SRC)
        dist_f = np.ascontiguousarray(np.asarray(dist_t, dtype=np.float32))
        alph_f = np.ascontiguousarray(np.asarray(alph_t, dtype=np.float32))
        for b in range(B):
            m = in_maps[b]
            del m["meta"]
            m["dist"] = dist_f[b]
            m["alpha"] = alph_f[b]
            m["ptr"] = np.ascontiguousarray(ptr[b].reshape(L_SRC, 1))
        variant = "dense"
    run.last_variant = variant
    res = None
    for attempt in range(3):
        try:
            res = run_bass_kernel_spmd(
                _get_nc(variant), in_maps, list(range(NCORES)),
                trace=trace and attempt == 0, **spmd_kwargs
            )
            break
        except Exception:
            # transient device-state failures (e.g. NRT_EXEC_UNIT_UNRECOVERABLE
            # left over from a previous profiled session) sometimes clear on
            # retry; give it two more chances (untraced -- profiling itself
            # can be the destabilizer) before giving up
            if attempt == 2:
                raise
            import time

            time.sleep(2.0)
    out = np.stack(
        [np.asarray(res.results[b]["out"]).astype(np.float32) for b in range(B)],
        axis=0,
    )
    return out, res


def kernel(dist_t, p_gen, alph_t, batch_vocab, pointer):
    out, _ = run(dist_t, p_gen, alph_t, batch_vocab, pointer)
    return out
